# revision 24
# baseline (speedup 1.0000x reference)
"""Trainium2 Bass kernel for the MAB-style dense transformer block.

Reference computation (per batch b of 4, channel-major [D=512, S=2048]):
    q = Wq @ Q + bq                  # [D, Sq]
    k = Wk @ K + bk                  # [D, Sk]
    v = Wv @ K + bv                  # [D, Sk]
    per head h (8 heads x 64 ch):
      logits = (q_h * 0.125)^T k_h   # [Sq, Sk]
      w = softmax(logits, axis=-1)
      att_h = (w @ v_h^T)^T          # [64, Sq]
    x  = q + att                     # residual
    y  = LN_ch(x; g0, b0)            # layernorm over channels
    z  = y + relu(Wo @ y + bo)
    out = LN_ch(z; g1, b1)

Sharding: 8 cores = batch (4) x query-half (2). Each core handles
[D, 1024] of queries for one batch with the full K/V — zero
cross-core communication.

V2 design (measured HW rates: matmul ~0.52ns/col + ~50ns/inst for both
f32r and bf16; ACT exp ~1.6us per [128,1024] regardless of dtype; DVE
~1.5ns/col f32 out, ~1.0ns/col bf16 out; Pool ~2.8ns/col):
  - all matmul inputs bf16 (halves DMA + SBUF, speeds DVE evacuations)
  - attention is software-pipelined across the flat (head, k-block)
    stream: logits(i+1) is emitted before AV(i), so the PE stays busy
    while ACT does exp(i) — the attention phase is ACT(exp)-bound at
    ~1.6us/step instead of the serial ~2.9us/step chain.
  - softmax denominator from a ones-column in the AV matmul (M=65);
    reciprocal read directly from PSUM; even head's normalize writes
    straight into the pair tile (one SBUF->SBUF DMA per pair, odd
    head only).
  - channel layernorm via ones-column matmuls for stats and K<=2
    broadcast matmuls for per-column scale/shift; squares + residual
    adds on Pool (GpSimd), everything else elementwise on DVE.
"""

import sys

sys.path.insert(0, "/opt/trn_rl_repo")

from contextlib import ExitStack

import numpy as np

import concourse.bass as bass
import concourse.tile as tile
from concourse import bacc, mybir
from concourse.bass_utils import run_bass_kernel_spmd

F32 = mybir.dt.float32
F32R = mybir.dt.float32r
BF16 = mybir.dt.bfloat16

B, D, H, DK = 4, 512, 8, 64
SQ, SK = 2048, 2048
QC = SQ // 2          # per-core query columns
CB = D // 128         # channel blocks of 128
KB = SK // 128        # key blocks of 128
NCH = 512             # matmul moving-dim chunk
SCALE = DK ** -0.5
EPS = 1e-12


def emit_core_kernel(ctx: ExitStack, tc: tile.TileContext, ins: dict, out_ap: bass.AP,
                     upto: str = "full"):
    nc = tc.nc
    EXP = mybir.ActivationFunctionType.Exp
    SQRT = mybir.ActivationFunctionType.Sqrt
    ADD = mybir.AluOpType.add
    MULT = mybir.AluOpType.mult

    p_const = ctx.enter_context(tc.tile_pool(name="const", bufs=1))
    p_persist = ctx.enter_context(tc.tile_pool(name="persist", bufs=1))
    # PSUM: 8 banks, all [.,512] f32 (1 bank per slot): big x4 + av x4.
    ps_big = ctx.enter_context(tc.tile_pool(name="psbig", bufs=4, space="PSUM"))
    ps_av = ctx.enter_context(tc.tile_pool(name="psav", bufs=4, space="PSUM"))

    # ---- constants ----
    woT = [p_const.tile([128, D], BF16, tag=f"woT{ci}", name=f"woT{ci}") for ci in range(CB)]
    for ci in range(CB):
        nc.sync.dma_start(out=woT[ci], in_=ins["WoT"][ci * 128:(ci + 1) * 128, :])

    def load_col_vec(name):
        # [512] dram -> [128, CB] sbuf, channel c at (partition c%128, col c//128)
        t = p_const.tile([128, CB], F32, tag=name)
        nc.sync.dma_start(out=t, in_=ins[name].rearrange("(m p) -> p m", p=128))
        return t

    bq_pp = load_col_vec("bq")
    bk_pp = load_col_vec("bk")
    bo_pp = load_col_vec("bo")

    bv_bc = p_const.tile([128, D], F32, tag="bv_bc", name="bv_bc")
    bv_in = ins["bv"]
    nc.sync.dma_start(
        out=bv_bc,
        in_=bass.AP(tensor=bv_in.tensor, offset=bv_in.offset,
                    ap=[[0, 128]] + bv_in.ap),
    )

    def load_gb(gname, bname, tag):
        t = p_const.tile([2, D], BF16, tag=tag)
        nc.sync.dma_start(out=t[0:1, :], in_=ins[gname][None, :])
        nc.sync.dma_start(out=t[1:2, :], in_=ins[bname][None, :])
        return t

    gb0 = load_gb("g0", "b0", "gb0")
    gb1 = load_gb("g1", "b1", "gb1")

    ones_in = ins["ones_c"]  # [128, 8] of ones, bf16
    oned_col = p_const.tile([128, 1], BF16, tag="oned_col", name="oned_col")
    nc.vector.memset(oned_col, 1.0 / D)  # 2^-9, exact in bf16
    onesT = p_const.tile([65, 64], BF16, tag="onesT", name="onesT")
    nc.sync.dma_start(out=onesT[64:65, :], in_=ins["ones_q"][None, 0:64])
    eps_t = p_const.tile([1, 1], F32, tag="eps", name="eps")
    nc.vector.memset(eps_t, EPS)

    # ---- phase 1: staging + v projection; q/k projections become blocks ----
    qch = [p_persist.tile([128, QC], BF16, tag=f"qch{m}", name=f"qch{m}") for m in range(CB)]
    kch = [p_persist.tile([128, SK], BF16, tag=f"kch{m}", name=f"kch{m}") for m in range(CB)]
    vs = [p_persist.tile([128, H, DK + 1], BF16, tag=f"vs{sb}", name=f"vs{sb}") for sb in range(KB)]
    # zero-padded per-head query tiles: the logits matmul then contracts over
    # K=128 (full rate; K=64 measured ~1.6x slower per column), with the other
    # head's channel rows zeroed so the extra products vanish.
    qzA = [p_persist.tile([128, QC], BF16, tag=f"qzA{m}", name=f"qzA{m}") for m in range(CB)]
    qzB = [p_persist.tile([128, QC], BF16, tag=f"qzB{m}", name=f"qzB{m}") for m in range(CB)]
    for m in range(CB):
        nc.gpsimd.memset(qzA[m][DK:128, :], 0.0)
        nc.gpsimd.memset(qzB[m][0:DK, :], 0.0)

    # stage pool stays open through attention: m=1..3 q/k projections are
    # interleaved into the attention stream's PE slack.
    p_stage = ctx.enter_context(tc.tile_pool(name="stage", bufs=1))
    wqT = [p_stage.tile([128, D], BF16, tag=f"wqT{ci}", name=f"wqT{ci}") for ci in range(CB)]
    wkT = [p_stage.tile([128, D], BF16, tag=f"wkT{ci}", name=f"wkT{ci}") for ci in range(CB)]
    wvT = [p_stage.tile([128, D], BF16, tag=f"wvT{ci}", name=f"wvT{ci}") for ci in range(CB)]
    qc = [p_stage.tile([128, QC], BF16, tag=f"qc{ci}", name=f"qc{ci}") for ci in range(CB)]
    kc = [p_stage.tile([128, SK], BF16, tag=f"kc{ci}", name=f"kc{ci}") for ci in range(CB)]
    for ci in range(CB):
        sl = slice(ci * 128, (ci + 1) * 128)
        nc.sync.dma_start(out=kc[ci], in_=ins["Kc"][sl, :])
        nc.sync.dma_start(out=wvT[ci], in_=ins["WvT"][sl, :])
        nc.sync.dma_start(out=qc[ci], in_=ins["Qc"][sl, :])
        nc.sync.dma_start(out=wqT[ci], in_=ins["WqT"][sl, :])
        nc.sync.dma_start(out=wkT[ci], in_=ins["WkT"][sl, :])

    def q_proj_block(m, n0):
        # one [128, 512] output chunk: 4 accumulating matmuls + bias evac
        mcols = slice(m * 128, (m + 1) * 128)
        ps = ps_big.tile([128, NCH], F32, tag="big", name="qproj")
        for ci in range(CB):
            nc.tensor.matmul(
                out=ps, lhsT=(wqT[ci][:, mcols]), rhs=(qc[ci][:, n0:n0 + NCH]),
                start=(ci == 0), stop=(ci == CB - 1),
            )
        nc.vector.tensor_scalar(
            out=qch[m][:, n0:n0 + NCH], in0=ps,
            scalar1=bq_pp[:, m:m + 1], scalar2=None, op0=ADD)
        nc.vector.tensor_copy(qzA[m][0:DK, n0:n0 + NCH],
                              qch[m][0:DK, n0:n0 + NCH])
        nc.vector.tensor_copy(qzB[m][DK:128, n0:n0 + NCH],
                              qch[m][DK:128, n0:n0 + NCH])

    def k_proj_block(m, n0):
        mcols = slice(m * 128, (m + 1) * 128)
        ps = ps_big.tile([128, NCH], F32, tag="big", name="kproj")
        for ci in range(CB):
            nc.tensor.matmul(
                out=ps, lhsT=(wkT[ci][:, mcols]), rhs=(kc[ci][:, n0:n0 + NCH]),
                start=(ci == 0), stop=(ci == CB - 1),
            )
        nc.vector.tensor_scalar(
            out=kch[m][:, n0:n0 + NCH], in0=ps,
            scalar1=bk_pp[:, m:m + 1], scalar2=None, op0=ADD)

    # v projection up front (first AV needs vs; sequence-major + ones col)
    for sb in range(KB):
        ps = ps_av.tile([128, NCH], F32, tag="av", name="vproj")
        for ci in range(CB):
            nc.tensor.matmul(
                out=ps,
                lhsT=(kc[ci][:, sb * 128:(sb + 1) * 128]),
                rhs=(wvT[ci][:, 0:D]),
                start=(ci == 0), stop=(ci == CB - 1),
            )
        nc.vector.tensor_add(
            vs[sb][:, :, 0:DK],
            ps.rearrange("p (h d) -> p h d", h=H),
            bv_bc.rearrange("p (h d) -> p h d", h=H),
        )
        nc.sync.dma_start(out=vs[sb][:, :, DK:DK + 1],
                          in_=ins["ones_c"][:, :, None])

    # m=0 k/q projections next (head pair 0 reads them immediately)
    for n0 in range(0, SK, NCH):
        k_proj_block(0, n0)
    for n0 in range(0, QC, NCH):
        q_proj_block(0, n0)

    # deferred projection blocks for m=1..3, drained during attention
    proj_blocks = []
    for m in range(1, CB):
        for n0 in range(0, SK, NCH):
            proj_blocks.append((k_proj_block, m, n0))
        for n0 in range(0, QC, NCH):
            proj_blocks.append((q_proj_block, m, n0))

    if upto == "proj":
        for fn, m, n0 in proj_blocks:
            fn(m, n0)
        for m in range(CB):
            nc.gpsimd.dma_start(out=out_ap[m * 128:(m + 1) * 128, :],
                                in_=qch[m][:, :])
        return

    # ---- phase 2: attention, software-pipelined over (head, kb, chunk) ----
    p_exp = ctx.enter_context(tc.tile_pool(name="exp", bufs=8))
    p_att = ctx.enter_context(tc.tile_pool(name="att", bufs=2))
    p_rec = ctx.enter_context(tc.tile_pool(name="rec", bufs=2))
    p_xz = ctx.enter_context(tc.tile_pool(name="xz", bufs=5))
    p_work = ctx.enter_context(tc.tile_pool(name="work", bufs=4))
    p_sq = ctx.enter_context(tc.tile_pool(name="sq", bufs=4))
    p_tmp = ctx.enter_context(tc.tile_pool(name="tmp", bufs=2))
    p_small = ctx.enter_context(tc.tile_pool(name="small", bufs=4))

    x = [None] * CB        # channel-major q+att blocks
    sqx = [None] * CB      # squared x blocks (for LN0 stats)
    attb = [None] * CB     # assembled attention pairs
    avc_map = {}

    deferred = {}  # unit index -> [callable]: run after that unit's L/exp

    def schedule(ui, fn):
        deferred.setdefault(ui, []).append(fn)

    def finish_head_a(hp, par, h):
        # part A (DVE only, emitted inline at the head's last AV):
        # reciprocal of the softmax denominator rows straight from PSUM
        av = avc_map[(hp, par)]
        recs = p_rec.tile([65, QC], BF16, tag="recs", name="recs")
        for ci, n0 in enumerate(range(0, QC, NCH)):
            nc.vector.reciprocal(recs[64:65, n0:n0 + NCH], av[ci][64:65, :])
        return recs

    def finish_head_b(hp, par, h, recs):
        # part B (deferred a few units so the PE rbc matmuls never wait on
        # the DVE reciprocal while at the front of the in-order PE queue)
        av = avc_map[(hp, par)]
        rbc_s = p_tmp.tile([DK, QC], BF16, tag="rbcs", name="rbc_s")
        for n0 in range(0, QC, NCH):
            rbcp = ps_big.tile([128, NCH], F32, tag="big", name="rbcp")
            nc.tensor.matmul(
                out=rbcp[0:DK, :],
                lhsT=(onesT[64:65, 0:DK]),
                rhs=(recs[64:65, n0:n0 + NCH]),
                start=True, stop=True,
            )
            nc.vector.tensor_copy(rbc_s[:, n0:n0 + NCH], rbcp[0:DK, :])
        if par == 0:
            attb[hp] = p_att.tile([128, QC], BF16, tag="attb", name="attb")
            for ci, n0 in enumerate(range(0, QC, NCH)):
                nc.vector.tensor_mul(attb[hp][0:DK, n0:n0 + NCH],
                                     av[ci][0:DK, :], rbc_s[:, n0:n0 + NCH])
        else:
            a_t = p_tmp.tile([DK, QC], BF16, tag="at", name="a_t")
            for ci, n0 in enumerate(range(0, QC, NCH)):
                nc.vector.tensor_mul(a_t[:, n0:n0 + NCH],
                                     av[ci][0:DK, :], rbc_s[:, n0:n0 + NCH])
            nc.sync.dma_start(out=attb[hp][DK:128, :], in_=a_t)
            xm = p_xz.tile([128, QC], BF16, tag="xz", name="xz")
            nc.gpsimd.tensor_add(xm, attb[hp], qch[hp])
            x[hp] = xm
            sq_t = p_sq.tile([128, QC], BF16, tag="sq", name="sq")
            nc.gpsimd.tensor_mul(sq_t, xm, xm)
            sqx[hp] = sq_t

    def emit_av(cur_ui, hp, par, h, kb, ci, et):
        av = avc_map[(hp, par)]
        nc.tensor.matmul(
            out=av[ci],
            lhsT=(vs[kb][:, h, :]),
            rhs=(et),
            start=(kb == 0), stop=(kb == KB - 1),
        )
        if kb == KB - 1 and ci == QC // NCH - 1:
            recs = finish_head_a(hp, par, h)
            schedule(cur_ui + 6,
                     lambda hp=hp, par=par, h=h, recs=recs:
                     finish_head_b(hp, par, h, recs))

    units = [(hp, par, h, kb, ci)
             for hp in range(H // 2)
             for par, h in ((0, 2 * hp), (64, 2 * hp + 1))
             for kb in range(KB)
             for ci in range(QC // NCH)]
    LAG = 4
    PROJ_EVERY = 10
    queue = []
    next_proj = 0
    for ui, (hp, par, h, kb, ci) in enumerate(units):
        m = hp
        n0 = ci * NCH
        if kb == 0 and ci == 0:
            avc_map[(hp, par)] = [
                ps_av.tile([DK + 1, NCH], F32, tag="av", name="avc")
                for _ in range(QC // NCH)]
        lps = ps_big.tile([128, NCH], F32, tag="big", name="lps")
        qz = qzA[m] if par == 0 else qzB[m]
        nc.tensor.matmul(
            out=lps,
            lhsT=(kch[m][:, kb * 128:(kb + 1) * 128]),
            rhs=(qz[:, n0:n0 + NCH]),
            start=True, stop=True,
        )
        et = p_exp.tile([128, NCH], BF16, tag="exp", name="exp")
        nc.scalar.activation(et, lps, EXP, bias=0.0, scale=SCALE)
        queue.append((hp, par, h, kb, ci, et))
        if len(queue) > LAG:
            emit_av(ui, *queue.pop(0))
        for fn in deferred.pop(ui, ()):
            fn()
        if ui % PROJ_EVERY == PROJ_EVERY - 1 and next_proj < len(proj_blocks):
            fn, pm, pn0 = proj_blocks[next_proj]
            fn(pm, pn0)
            next_proj += 1
    nu = len(units)
    while queue:
        emit_av(nu, *queue.pop(0))
    for ui in sorted(deferred):
        for fn in deferred.pop(ui):
            fn()
    while next_proj < len(proj_blocks):
        fn, pm, pn0 = proj_blocks[next_proj]
        fn(pm, pn0)
        next_proj += 1

    if upto == "attn":
        for m in range(CB):
            nc.gpsimd.dma_start(out=out_ap[m * 128:(m + 1) * 128, :],
                                in_=x[m][:, :])
        return

    # ---- phase 3: tail (LN0 -> conv+relu residual -> LN1) ----
    def ln_stats(blocks, sqs):
        """Stats for channel-axis LN. Returns (rstd_bf, rhsB) bf16 rows.
        lhsT is pre-scaled by 1/D so PSUM holds mean / E[x^2] directly."""
        mean_ps = [ps_av.tile([1, NCH], F32, tag="av", name="mean_ps")
                   for _ in range(QC // NCH)]
        ex2_ps = [ps_av.tile([1, NCH], F32, tag="av", name="ex2_ps")
                  for _ in range(QC // NCH)]
        for ci, n0 in enumerate(range(0, QC, NCH)):
            for m in range(CB):
                nc.tensor.matmul(
                    out=mean_ps[ci],
                    lhsT=(oned_col),
                    rhs=(blocks[m][:, n0:n0 + NCH]),
                    start=(m == 0), stop=(m == CB - 1),
                )
        for ci, n0 in enumerate(range(0, QC, NCH)):
            for m in range(CB):
                nc.tensor.matmul(
                    out=ex2_ps[ci],
                    lhsT=(oned_col),
                    rhs=(sqs[m][:, n0:n0 + NCH]),
                    start=(m == 0), stop=(m == CB - 1),
                )

        mean_s = p_small.tile([1, QC], F32, tag="stat", name="mean_s")
        for ci, n0 in enumerate(range(0, QC, NCH)):
            nc.vector.tensor_copy(mean_s[:, n0:n0 + NCH], mean_ps[ci])
        m2 = p_small.tile([1, QC], F32, tag="stat", name="m2")
        nc.scalar.square(m2, mean_s)
        var = p_small.tile([1, QC], F32, tag="stat", name="var")
        for ci, n0 in enumerate(range(0, QC, NCH)):
            nc.vector.tensor_sub(var[:, n0:n0 + NCH], ex2_ps[ci],
                                 m2[:, n0:n0 + NCH])
        sd = p_small.tile([1, QC], F32, tag="stat", name="sd")
        nc.scalar.activation(sd, var, SQRT, bias=eps_t, scale=1.0)
        rstd_bf = p_small.tile([1, QC], BF16, tag="statb", name="rstd_bf")
        nc.vector.reciprocal(rstd_bf, sd)
        # rhsB rows: [0] = -mean*rstd, [1] = ones
        rhsB = p_small.tile([2, QC], BF16, tag="statb", name="rhsB")
        nc.sync.dma_start(out=rhsB[1:2, :], in_=ins["ones_q"][None, :])
        nc.vector.scalar_tensor_tensor(
            out=rhsB[0:1, :], in0=mean_s, scalar=-1.0, in1=rstd_bf,
            op0=MULT, op1=MULT)
        return rstd_bf, rhsB

    def ln_apply(blocks, m, gb, rstd_bf, rhsB, out_pool, out_tag, out_dt):
        mcols = slice(m * 128, (m + 1) * 128)
        o = out_pool.tile([128, QC], out_dt, tag=out_tag)
        t = p_tmp.tile([128, QC], BF16, tag="tmp", name="lnt")
        for n0 in range(0, QC, NCH):
            a_ps = ps_big.tile([128, NCH], F32, tag="big", name="a_ps")
            nc.tensor.matmul(
                out=a_ps, lhsT=(gb[0:1, mcols]), rhs=(rstd_bf[:, n0:n0 + NCH]),
                start=True, stop=True,
            )
            nc.vector.tensor_mul(t[:, n0:n0 + NCH],
                                 blocks[m][:, n0:n0 + NCH], a_ps)
        for n0 in range(0, QC, NCH):
            b_ps = ps_big.tile([128, NCH], F32, tag="big", name="b_ps")
            nc.tensor.matmul(
                out=b_ps, lhsT=(gb[0:2, mcols]), rhs=(rhsB[:, n0:n0 + NCH]),
                start=True, stop=True,
            )
            nc.vector.tensor_add(o[:, n0:n0 + NCH], t[:, n0:n0 + NCH], b_ps)
        return o

    rstd0, rhsB0 = ln_stats(x, sqx)
    y0 = [ln_apply(x, m, gb0, rstd0, rhsB0, p_work, "work", BF16)
          for m in range(CB)]

    # conv per output block, with relu/residual/LN1-square overlapping the
    # next block's matmuls; LN1 sum/sq chains accumulate as z blocks land.
    z = []
    sqz = []
    for o in range(CB):
        ocols = slice(o * 128, (o + 1) * 128)
        r_t = p_tmp.tile([128, QC], BF16, tag="tmp", name="relu")
        for n0 in range(0, QC, NCH):
            cps = ps_big.tile([128, NCH], F32, tag="big", name="cps")
            for ci in range(CB):
                nc.tensor.matmul(
                    out=cps,
                    lhsT=(woT[ci][:, ocols]),
                    rhs=(y0[ci][:, n0:n0 + NCH]),
                    start=(ci == 0), stop=(ci == CB - 1),
                )
            nc.scalar.activation(r_t[:, n0:n0 + NCH], cps,
                                 mybir.ActivationFunctionType.Relu,
                                 bias=bo_pp[:, o:o + 1], scale=1.0)
        zo = p_xz.tile([128, QC], BF16, tag="xz", name="xz")
        nc.vector.tensor_add(zo, r_t, y0[o])
        z.append(zo)
        sq_t = p_sq.tile([128, QC], BF16, tag="sq", name="sqz")
        nc.vector.tensor_mul(sq_t, zo, zo)
        sqz.append(sq_t)

    rstd1, rhsB1 = ln_stats(z, sqz)
    for m in range(CB):
        fin = ln_apply(z, m, gb1, rstd1, rhsB1, p_xz, "fin", F32R)
        nc.sync.dma_start(out=out_ap[m * 128:(m + 1) * 128, :], in_=fin)


OUT_DT = F32R


def declare_inputs(nc):
    ins = {}
    ins["Qc"] = nc.dram_tensor("Qc", [D, QC], BF16, kind="ExternalInput").ap()
    ins["Kc"] = nc.dram_tensor("Kc", [D, SK], BF16, kind="ExternalInput").ap()
    for w in ("WqT", "WkT", "WvT", "WoT"):
        ins[w] = nc.dram_tensor(w, [D, D], BF16, kind="ExternalInput").ap()
    for vname in ("bq", "bk", "bv", "bo"):
        ins[vname] = nc.dram_tensor(vname, [D], F32, kind="ExternalInput").ap()
    for vname in ("g0", "b0", "g1", "b1"):
        ins[vname] = nc.dram_tensor(vname, [D], BF16, kind="ExternalInput").ap()
    ins["ones_c"] = nc.dram_tensor("ones_c", [128, H], BF16,
                                   kind="ExternalInput").ap()
    ins["ones_q"] = nc.dram_tensor("ones_q", [QC], BF16,
                                   kind="ExternalInput").ap()
    return ins


def make_in_maps(inputs):
    """Full-input dict (reference naming) -> 8 per-core input maps."""
    import ml_dtypes

    bf = ml_dtypes.bfloat16
    Q = np.asarray(inputs["Q"], dtype=np.float32).astype(bf)
    Kf = np.asarray(inputs["K"], dtype=np.float32).astype(bf)
    shared = {
        "WqT": np.ascontiguousarray(np.asarray(inputs["Wq"], np.float32).T).astype(bf),
        "WkT": np.ascontiguousarray(np.asarray(inputs["Wk"], np.float32).T).astype(bf),
        "WvT": np.ascontiguousarray(np.asarray(inputs["Wv"], np.float32).T).astype(bf),
        "WoT": np.ascontiguousarray(np.asarray(inputs["Wo"], np.float32).T).astype(bf),
        "bq": np.asarray(inputs["bq"], np.float32),
        "bk": np.asarray(inputs["bk"], np.float32),
        "bv": np.asarray(inputs["bv"], np.float32),
        "bo": np.asarray(inputs["bo"], np.float32),
        "g0": np.asarray(inputs["gamma0"], np.float32).astype(bf),
        "b0": np.asarray(inputs["beta0"], np.float32).astype(bf),
        "g1": np.asarray(inputs["gamma1"], np.float32).astype(bf),
        "b1": np.asarray(inputs["beta1"], np.float32).astype(bf),
        "ones_c": np.ones((128, H), dtype=bf),
        "ones_q": np.ones((QC,), dtype=bf),
    }
    in_maps = []
    for core in range(8):
        b, j = core // 2, core % 2
        m = dict(shared)
        m["Qc"] = np.ascontiguousarray(Q[b, :, j * QC:(j + 1) * QC])
        m["Kc"] = np.ascontiguousarray(Kf[b])
        in_maps.append(m)
    return in_maps


def build_module():
    nc = bacc.Bacc("TRN2", target_bir_lowering=False, debug=False)
    ins = declare_inputs(nc)
    out_ap = nc.dram_tensor("out", [D, QC], OUT_DT, kind="ExternalOutput").ap()

    with tile.TileContext(nc) as tc:
        with nc.allow_low_precision(reason="bf16 tiles feed full-rate matmuls"):
            with ExitStack() as ctx:
                emit_core_kernel(ctx, tc, ins, out_ap)
    nc.compile()
    return nc


_NC_CACHE = None


def _get_nc():
    global _NC_CACHE
    if _NC_CACHE is None:
        _NC_CACHE = build_module()
    return _NC_CACHE


def kernel(Q, K, Wq, bq, Wk, bk, Wv, bv, Wo, bo, gamma0, beta0, gamma1, beta1,
           _trace=False, _trace_cores=None):
    in_maps = make_in_maps({
        "Q": Q, "K": K, "Wq": Wq, "Wk": Wk, "Wv": Wv, "Wo": Wo,
        "bq": bq, "bk": bk, "bv": bv, "bo": bo,
        "gamma0": gamma0, "beta0": beta0, "gamma1": gamma1, "beta1": beta1,
    })

    nc = _get_nc()
    res = run_bass_kernel_spmd(
        nc, in_maps, core_ids=list(range(8)),
        trace=_trace, trace_cores=_trace_cores,
    )
    out = np.empty((B, D, SQ), dtype=np.float32)
    for core in range(8):
        b, j = core // 2, core % 2
        out[b, :, j * QC:(j + 1) * QC] = res.results[core]["out"]
    if _trace:
        kernel._last_result = res
    return out
